# revision 20
# baseline (speedup 1.0000x reference)
"""Trainium2 Bass kernel for the additive coupling flow (nn_Additive_flow).

Math: 65 sequential steps. Step s (i = idx[s]) updates column i of z:
    z[:, i] += MLP_s(z with cols i<->63 swapped, first 63 cols) + b3[s]
Reformulated with no data permutation:
    h1 = relu(z @ W1e[s])      W1e[s] = [W1[s]; 0] with rows i,63 swapped
    h2 = relu(h1 @ W2[s])
    z[:, i] += h2 @ w3[s]      (plus biases; zero in practice)
Finally out = exp(s_vec) * z.

Device layout ("split-sample"): each core's 16384 samples are split in two
halves of 8192; SBUF state z is [128, 8192] bf16 where partitions 0:63
hold features of half A and 64:127 of half B. Every matmul runs in the
uniform 128x128 PE array mode (tiling-mode switches drain the PE):
  - L1 uses block-diagonal weight tiles [W1e_c;0] / [0;W1e_c] so each
    half's h1 is a full-K matmul over the shared z tile (4 MMs/macro).
  - L2 is 8 MMs/macro (2 K-chunks x 2 M-chunks x 2 halves).
  - L3 uses M=128 zero-padded scatter tiles (w3 in column idx[s], halves
    in column ranges 0:64 / 64:128) so all 4 MMs accumulate into ONE PSUM
    bank; the z update is a single [128, 512] DVE add per 1024 samples.
Emission is software-pipelined (stage A = L1+relu one macro ahead of
stage B = L2+relu+L3+zadd) so the per-engine FIFO order interleaves
macros; PSUM pools (8+6+2 KB) fit the 16KB budget exactly. Weights stream
per step through a rotating pool with 2-step prefetch. Relus are split
across ACT and DVE to balance engine time. Measured: tensor engine ~99%
active, ~4.45 ms (vs 5.18 ms baseline). fp8/DoubleRow was evaluated and
rejected: the 65-step feedback amplifies e4m3 quantization to ~0.44 rel
err (budget 2e-2); bf16 gives 3.0e-3.
"""

import os
import sys

for _p in ("/opt/trn_rl_repo", "/root/.axon_site/_ro/trn_rl_repo"):
    if os.path.isdir(_p) and _p not in sys.path:
        sys.path.append(_p)

import numpy as np
import concourse.bass as bass
import concourse.bacc as bacc
import concourse.mybir as mybir
from concourse.tile import TileContext
from concourse.bass_utils import run_bass_kernel_spmd

NCORES = 8
B = 131072
N = 64          # latent dim
S = 65          # coupling steps
H = 256         # MLP width
BSH = B // NCORES      # 16384 samples per core
HALF = BSH // 2        # 8192 samples per partition-half
FD = 512               # matmul moving free-dim; one macro = 2*FD samples
NMACRO = HALF // FD    # 16
WCOLS = 1536           # per-step packed weight columns

F32 = mybir.dt.float32
F32R = mybir.dt.float32r
BF16 = mybir.dt.bfloat16
AF = mybir.ActivationFunctionType
ALU = mybir.AluOpType

LAST_RESULT = None  # test.py reads exec_time_ns from here

_HOOK_SRC = """\
_hook = None


def set_axon_ntff_profile_hook(h):
    global _hook
    _hook = h


def get_axon_ntff_profile_hook():
    return _hook
"""


def _ensure_ntff_hook():
    """Install the axon NTFF profiling hook if the image's antenv lacks
    axon_hooks (degrades silently; tracing is optional)."""
    try:
        from antenv.axon_hooks import get_axon_ntff_profile_hook

        if get_axon_ntff_profile_hook() is not None:
            return
        have_module = True
    except ImportError:
        have_module = False
    try:
        import antenv

        if not have_module:
            ext = "/tmp/axon_hooks_ext"
            os.makedirs(ext, exist_ok=True)
            p = os.path.join(ext, "axon_hooks.py")
            if not os.path.exists(p):
                with open(p, "w") as f:
                    f.write(_HOOK_SRC)
            if ext not in antenv.__path__:
                antenv.__path__.append(ext)
        from antenv.axon_hooks import (
            get_axon_ntff_profile_hook,
            set_axon_ntff_profile_hook,
        )

        if get_axon_ntff_profile_hook() is None:
            from trn_agent_boot.trn_boot import _ntff_profile_via_ctypes

            hook = _ntff_profile_via_ctypes("/opt/axon/libaxon_pjrt.so")
            if hook is not None:
                set_axon_ntff_profile_hook(hook)
    except Exception:
        pass


def build_program_v2(nsteps=S, nmacro=NMACRO):
    """v3: all matmuls in uniform 128x128 array mode (no tiling-mode drains).

    L1 uses block-diagonal weight tiles ([W1e;0] / [0;W1e]) so each half's
    h1 comes from a full-K matmul over the shared z tile. L3 uses M=128
    zero-padded W3e tiles so both halves' updates accumulate into one PSUM
    bank. Emission is software-pipelined: stage A (L1+relu) runs one macro
    ahead of stage B (L2+relu+L3+zadd). Weights stream per step through a
    rotating pool with 2-step prefetch.
    """
    half = nmacro * FD
    nc = bacc.Bacc("TRN2", target_bir_lowering=False, debug=False)

    xt = nc.dram_tensor("xt", [128, half], BF16, kind="ExternalInput")
    wq = nc.dram_tensor("wq", [128, nsteps * WCOLS], BF16, kind="ExternalInput")
    s_d = nc.dram_tensor("sv", [128, 1], F32, kind="ExternalInput")
    out_d = nc.dram_tensor("out", [128, half], F32, kind="ExternalOutput")

    with TileContext(nc) as tc:
        with (
            tc.tile_pool(name="zpool", bufs=1) as zp,
            tc.tile_pool(name="consts", bufs=1) as cp,
            tc.tile_pool(name="wpool", bufs=4) as wp,
            tc.tile_pool(name="h1pool", bufs=4) as h1p,
            tc.tile_pool(name="h2pool", bufs=4) as h2p,
            tc.tile_pool(name="opool", bufs=3) as op,
            tc.tile_pool(name="psH1", bufs=1, space="PSUM") as pH1,
            tc.tile_pool(name="psH2", bufs=1, space="PSUM") as pH2,
            tc.tile_pool(name="psD", bufs=1, space="PSUM") as pD,
        ):
            # --- constants ---
            ss = cp.tile([128, 1], F32, tag="ss")
            nc.sync.dma_start(ss[:], s_d[:])
            exps = cp.tile([128, 1], F32, tag="exps")
            nc.scalar.activation(exps[:], ss[:], AF.Exp)

            def fetch_weights(st):
                wt = wp.tile([128, WCOLS], BF16, tag="w")
                nc.sync.dma_start(wt[:], wq[:, bass.ts(st, WCOLS)])
                return wt

            # --- z state, resident in SBUF ---
            zt = zp.tile([128, half], BF16, tag="z")
            for m in range(nmacro):
                nc.gpsimd.dma_start(zt[:, bass.ts(m, FD)], xt[:, bass.ts(m, FD)])

            wts = {0: fetch_weights(0)}
            if nsteps > 1:
                wts[1] = fetch_weights(1)

            def stageA(st, m):
                """L1 for (st, m): 4 block-diag MMs + h1 relus."""
                wt = wts[st]
                zsl = zt[:, bass.ts(m, FD)]
                h1a = pH1.tile([128, 2 * FD], F32, tag="h1a")
                h1b = pH1.tile([128, 2 * FD], F32, tag="h1b")
                # tiles: [W1e_c0;0] [0;W1e_c0] [W1e_c1;0] [0;W1e_c1]
                nc.tensor.matmul(h1a[:, 0:FD], wt[:, 0:128], zsl)
                nc.tensor.matmul(h1b[:, 0:FD], wt[:, 128:256], zsl)
                nc.tensor.matmul(h1a[:, FD : 2 * FD], wt[:, 256:384], zsl)
                nc.tensor.matmul(h1b[:, FD : 2 * FD], wt[:, 384:512], zsl)
                h1wA = h1p.tile([128, 2 * FD], BF16, tag="h1wA")
                h1wB = h1p.tile([128, 2 * FD], BF16, tag="h1wB")
                nc.scalar.activation(h1wA[:], h1a[:], AF.Relu)
                nc.vector.tensor_scalar(h1wB[:], h1b[:], 0.0, None, op0=ALU.max)
                return h1wA, h1wB

            def stageB(st, m, h1w):
                """L2 + h2 relus + L3 (M=128-padded, one bank) + z add."""
                wt = wts[st]
                h1wA, h1wB = h1w
                zsl = zt[:, bass.ts(m, FD)]
                w2 = [wt[:, 512 + 128 * i : 640 + 128 * i] for i in range(4)]
                w3 = [wt[:, 1024 + 128 * i : 1152 + 128 * i] for i in range(4)]

                h2ts = []
                h2w = {}
                for hi, (hw, hx) in enumerate(((h1wA, "A"), (h1wB, "B"))):
                    for mc in range(2):
                        if hi == 1 and mc == 1:
                            ps = h2ts[0]  # reuse A-m0 bank (PSUM budget)
                        else:
                            ps = pH2.tile([128, FD], F32, tag=f"h2_{len(h2ts)}")
                            h2ts.append(ps)
                        nc.tensor.matmul(
                            ps[:], w2[2 * mc], hw[:, 0:FD], start=True, stop=False
                        )
                        nc.tensor.matmul(
                            ps[:], w2[2 * mc + 1], hw[:, FD : 2 * FD],
                            start=False, stop=True,
                        )
                        if mc == 0:
                            h2wt = h2p.tile([128, 2 * FD], BF16, tag=f"h2w{hx}")
                            h2w[hx] = h2wt
                        osl = h2w[hx][:, mc * FD : (mc + 1) * FD]
                        if hi == 1 and mc == 1:
                            nc.vector.tensor_scalar(
                                osl, ps[:], 0.0, None, op0=ALU.max
                            )
                        else:
                            nc.scalar.activation(osl, ps[:], AF.Relu)

                dps = pD.tile([128, FD], F32, tag="dps")
                nc.tensor.matmul(
                    dps[:], w3[0], h2w["A"][:, 0:FD], start=True, stop=False
                )
                nc.tensor.matmul(
                    dps[:], w3[1], h2w["A"][:, FD : 2 * FD], start=False, stop=False
                )
                nc.tensor.matmul(
                    dps[:], w3[2], h2w["B"][:, 0:FD], start=False, stop=False
                )
                nc.tensor.matmul(
                    dps[:], w3[3], h2w["B"][:, FD : 2 * FD], start=False, stop=True
                )
                nc.vector.tensor_add(zsl, dps[:], zsl)
                if st == nsteps - 1:
                    # last step: scale + store this macro right away so the
                    # output phase overlaps the remaining macros' compute
                    ostage = op.tile([128, FD], F32, tag="ostage")
                    nc.vector.tensor_scalar_mul(ostage[:], zsl, exps[:])
                    nc.sync.dma_start(out_d[:, bass.ts(m, FD)], ostage[:])

            # --- software-pipelined main loop: A one macro ahead of B ---
            flat = [(st, m) for st in range(nsteps) for m in range(nmacro)]
            pending = None  # (st, m, h1w)
            for k, (st, m) in enumerate(flat):
                # prefetch next step's weights mid-step
                if m == nmacro // 2 and st + 2 < nsteps:
                    wts[st + 2] = fetch_weights(st + 2)
                h1w = stageA(st, m)
                if pending is not None:
                    pst, pm, ph1w = pending
                    stageB(pst, pm, ph1w)
                pending = (st, m, h1w)
            pst, pm, ph1w = pending
            stageB(pst, pm, ph1w)

    nc.finalize()
    return nc


def host_prep_v2(x, s, W1, W2, W3, idx):
    """Build per-step packed weights and the split-sample transposed x."""
    import ml_dtypes

    W1 = np.asarray(W1, np.float32)
    W2 = np.asarray(W2, np.float32)
    W3 = np.asarray(W3, np.float32)
    idx = np.asarray(idx)

    wq = np.zeros((128, S * WCOLS), np.float32)
    for st in range(S):
        i = int(idx[st])
        W1e = np.zeros((N, H), np.float32)
        W1e[: N - 1] = W1[st]
        W1e[[i, N - 1]] = W1e[[N - 1, i]]
        W3e = np.zeros((H, N), np.float32)
        W3e[:, i] = W3[st, :, 0]
        w0 = st * WCOLS
        # L1 block-diagonal tiles: [W1e_c;0] for half A, [0;W1e_c] for B
        wq[0:64, w0 + 0 : w0 + 128] = W1e[:, 0:128]
        wq[64:128, w0 + 128 : w0 + 256] = W1e[:, 0:128]
        wq[0:64, w0 + 256 : w0 + 384] = W1e[:, 128:256]
        wq[64:128, w0 + 384 : w0 + 512] = W1e[:, 128:256]
        # W2 tiles (k-chunk-major within each m-chunk)
        wq[:, w0 + 512 : w0 + 640] = W2[st, 0:128, 0:128]
        wq[:, w0 + 640 : w0 + 768] = W2[st, 128:256, 0:128]
        wq[:, w0 + 768 : w0 + 896] = W2[st, 0:128, 128:256]
        wq[:, w0 + 896 : w0 + 1024] = W2[st, 128:256, 128:256]
        # W3e M=128-padded tiles: cols 0:64 update half A, 64:128 half B
        wq[:, w0 + 1024 : w0 + 1088] = W3e[0:128, :]
        wq[:, w0 + 1152 : w0 + 1216] = W3e[128:256, :]
        wq[:, w0 + 1344 : w0 + 1408] = W3e[0:128, :]
        wq[:, w0 + 1472 : w0 + 1536] = W3e[128:256, :]
    wq = wq.astype(ml_dtypes.bfloat16)

    sv = np.asarray(s, np.float32).reshape(N, 1)
    sv = np.concatenate([sv, sv], axis=0)  # [128, 1]
    return wq, sv


_PROGRAM_V2 = None
_RUN_IDX = 0


def kernel(x, s, W1, b1, W2, b2, W3, b3, idx):
    global LAST_RESULT, _PROGRAM_V2
    use_bias = bool(
        np.abs(b1).max() > 0 or np.abs(b2).max() > 0 or np.abs(b3).max() > 0
    )
    if use_bias:
        return _kernel_v1(x, s, W1, b1, W2, b2, W3, b3, idx)

    x = np.asarray(x, np.float32)
    wq, sv = host_prep_v2(x, s, W1, W2, W3, idx)
    in_maps = []
    for c in range(NCORES):
        xc = x[c * BSH : (c + 1) * BSH]
        xts = np.empty((128, HALF), np.float32)
        xts[0:64] = xc[0:HALF].T
        xts[64:128] = xc[HALF:BSH].T
        import ml_dtypes
        in_maps.append(
            dict(xt=np.ascontiguousarray(xts).astype(ml_dtypes.bfloat16),
                 wq=wq, sv=sv)
        )

    if _PROGRAM_V2 is None:
        _PROGRAM_V2 = build_program_v2()
    _ensure_ntff_hook()
    global _RUN_IDX
    tmpdir = os.environ.get("KERNEL_TMPDIR")
    if tmpdir:
        tmpdir = os.path.join(tmpdir, f"run{_RUN_IDX}")
        _RUN_IDX += 1
        os.makedirs(tmpdir, exist_ok=True)
    res = run_bass_kernel_spmd(
        _PROGRAM_V2, in_maps, core_ids=list(range(NCORES)), tmpdir=tmpdir
    )
    LAST_RESULT = res
    out = np.empty((B, N), np.float32)
    for c in range(NCORES):
        o = res.results[c]["out"]  # [128, HALF]
        out[c * BSH : c * BSH + HALF] = o[0:64].T
        out[c * BSH + HALF : (c + 1) * BSH] = o[64:128].T
    return out


# ---------------------------------------------------------------------------
# v1 fallback (baseline) — used only when biases are nonzero.
# ---------------------------------------------------------------------------
TILE = 512
MACRO = 1024
_PROGRAM_V1 = {}


def build_program_v1(nsteps=S, nmacro=BSH // MACRO, use_bias=True, hbufs=3):
    bsh = nmacro * MACRO
    nc = bacc.Bacc("TRN2", target_bir_lowering=False, debug=False)

    xt = nc.dram_tensor("xt", [N, bsh], BF16, kind="ExternalInput")
    wp_d = nc.dram_tensor("wpack", [nsteps, 128, 896], BF16, kind="ExternalInput")
    b1_d = nc.dram_tensor("b1r", [128, 2 * nsteps], F32, kind="ExternalInput")
    b2_d = nc.dram_tensor("b2r", [128, 2 * nsteps], F32, kind="ExternalInput")
    b3_d = nc.dram_tensor("b3c", [N, nsteps], F32, kind="ExternalInput")
    s_d = nc.dram_tensor("sv", [N, 1], F32, kind="ExternalInput")
    out_d = nc.dram_tensor("out", [N, bsh], F32, kind="ExternalOutput")

    with TileContext(nc) as tc:
        with (
            tc.tile_pool(name="zpool", bufs=1) as zp,
            tc.tile_pool(name="consts", bufs=1) as cp,
            tc.tile_pool(name="wpool", bufs=4) as wp,
            tc.tile_pool(name="hpool", bufs=hbufs) as hp,
            tc.tile_pool(name="psA", bufs=3, space="PSUM") as pA,
            tc.tile_pool(name="psB", bufs=3, space="PSUM") as pB,
            tc.tile_pool(name="psZ", bufs=2, space="PSUM") as pZ,
        ):
            if use_bias:
                b1s = cp.tile([128, 2 * nsteps], F32, tag="b1s")
                nc.sync.dma_start(b1s[:], b1_d[:])
                b2s = cp.tile([128, 2 * nsteps], F32, tag="b2s")
                nc.sync.dma_start(b2s[:], b2_d[:])
                b3s = cp.tile([N, nsteps], F32, tag="b3s")
                nc.sync.dma_start(b3s[:], b3_d[:])
            ss = cp.tile([N, 1], F32, tag="ss")
            nc.sync.dma_start(ss[:], s_d[:])
            exps = cp.tile([N, 1], F32, tag="exps")
            nc.scalar.activation(exps[:], ss[:], AF.Exp)

            def fetch_weights(st):
                wt = wp.tile([128, 896], BF16, tag="w")
                nc.sync.dma_start(wt[:], wp_d[st])
                return (
                    wt[0:N, 0:H], wt[:, 256:512], wt[:, 512:768],
                    wt[:, 768:832], wt[:, 832:896],
                )

            wtiles = fetch_weights(0)
            zt = zp.tile([N, bsh], BF16, tag="z")
            for m in range(nmacro):
                msl = bass.ts(m, MACRO)
                nc.gpsimd.dma_start(zt[:, msl], xt[:, msl])

            pending_l3 = None
            for st in range(nsteps):
                if st > 0:
                    wtiles = fetch_weights(st)
                w1t, w2ta, w2tb, w3ta, w3tb = wtiles

                for m in range(nmacro):
                    zsl = zt[:, bass.ts(m, MACRO)]

                    def act_relu(out, in_, bcol):
                        if use_bias:
                            nc.scalar.activation(out, in_, AF.Relu, bias=bcol)
                        else:
                            nc.scalar.activation(out, in_, AF.Relu)

                    def dve_relu(out, in_, bcol):
                        if use_bias:
                            nc.vector.tensor_scalar(
                                out, in_, bcol, 0.0, op0=ALU.add, op1=ALU.max
                            )
                        else:
                            nc.vector.tensor_scalar(out, in_, 0.0, None, op0=ALU.max)

                    b1a = b1s[:, 2 * st : 2 * st + 1] if use_bias else None
                    b1b = b1s[:, 2 * st + 1 : 2 * st + 2] if use_bias else None
                    b2a = b2s[:, 2 * st : 2 * st + 1] if use_bias else None
                    b2b = b2s[:, 2 * st + 1 : 2 * st + 2] if use_bias else None

                    h1ps = []
                    for t in range(MACRO // TILE):
                        tsl = bass.ts(t, TILE)
                        pa = pA.tile([128, TILE], F32, tag="h1p")
                        pb = pA.tile([128, TILE], F32, tag="h1p")
                        nc.tensor.matmul(pa[:], w1t[:, 0:128], zsl[:, tsl])
                        nc.tensor.matmul(pb[:], w1t[:, 128:256], zsl[:, tsl])
                        h1ps.append((pa, pb))
                    if pending_l3 is not None:
                        pending_l3()
                        pending_l3 = None
                    h1a = hp.tile([128, MACRO], BF16, tag="h1a")
                    h1b = hp.tile([128, MACRO], BF16, tag="h1b")
                    act_relu(h1a[:, 0:TILE], h1ps[0][0][:], b1a)
                    dve_relu(h1b[:, 0:TILE], h1ps[0][1][:], b1b)
                    act_relu(h1a[:, TILE:MACRO], h1ps[1][0][:], b1a)
                    act_relu(h1b[:, TILE:MACRO], h1ps[1][1][:], b1b)

                    h2a = hp.tile([128, MACRO], BF16, tag="h2a")
                    h2b = hp.tile([128, MACRO], BF16, tag="h2b")
                    for t in range(MACRO // TILE):
                        tsl = bass.ts(t, TILE)
                        pa = pB.tile([128, TILE], F32, tag="h2p")
                        pb = pB.tile([128, TILE], F32, tag="h2p")
                        nc.tensor.matmul(
                            pa[:], w2ta[:, 0:128], h1a[:, tsl], start=True, stop=False
                        )
                        nc.tensor.matmul(
                            pa[:], w2tb[:, 0:128], h1b[:, tsl], start=False, stop=True
                        )
                        nc.tensor.matmul(
                            pb[:], w2ta[:, 128:256], h1a[:, tsl], start=True, stop=False
                        )
                        nc.tensor.matmul(
                            pb[:], w2tb[:, 128:256], h1b[:, tsl], start=False, stop=True
                        )
                        act_relu(h2a[:, tsl], pa[:], b2a)
                        dve_relu(h2b[:, tsl], pb[:], b2b)

                    def emit_l3(h2a=h2a, h2b=h2b, zsl=zsl, w3ta=w3ta, w3tb=w3tb, st=st):
                        for t in range(MACRO // TILE):
                            tsl = bass.ts(t, TILE)
                            zps = pZ.tile([N, TILE], F32, tag="zp")
                            nc.tensor.matmul(
                                zps[:], w3ta[:], h2a[:, tsl], start=True, stop=False
                            )
                            nc.tensor.matmul(
                                zps[:], w3tb[:], h2b[:, tsl], start=False, stop=True
                            )
                            ztile = zsl[:, tsl]
                            if use_bias:
                                nc.vector.scalar_tensor_tensor(
                                    ztile, zps[:], b3s[:, st : st + 1], ztile,
                                    op0=ALU.add, op1=ALU.add,
                                )
                            else:
                                nc.vector.tensor_add(ztile, zps[:], ztile)

                    pending_l3 = emit_l3

            if pending_l3 is not None:
                pending_l3()

            for m in range(nmacro):
                msl = bass.ts(m, MACRO)
                ostage = hp.tile([N, MACRO], F32, tag="ostage")
                nc.vector.tensor_scalar_mul(ostage[:], zt[:, msl], exps[:])
                nc.sync.dma_start(out_d[:, msl], ostage[:])

    nc.finalize()
    return nc


def _host_prep_v1(x, s, W1, b1, W2, b2, W3, b3, idx, nsteps=S):
    x = np.asarray(x, np.float32)
    idx = np.asarray(idx)
    W1 = np.asarray(W1, np.float32)
    W2 = np.ascontiguousarray(np.asarray(W2, np.float32)[:nsteps])
    W3 = np.asarray(W3, np.float32)
    b1 = np.asarray(b1, np.float32)
    b2 = np.asarray(b2, np.float32)
    b3 = np.asarray(b3, np.float32)

    W1e = np.zeros((nsteps, N, H), np.float32)
    W1e[:, : N - 1, :] = W1[:nsteps]
    for st in range(nsteps):
        i = int(idx[st])
        r = W1e[st].copy()
        r[[i, N - 1]] = r[[N - 1, i]]
        W1e[st] = r
    W3e = np.zeros((nsteps, H, N), np.float32)
    for st in range(nsteps):
        W3e[st, :, int(idx[st])] = W3[st, :, 0]
    b3c = np.zeros((N, nsteps), np.float32)
    for st in range(nsteps):
        b3c[int(idx[st]), st] = b3[st, 0]
    import ml_dtypes
    wpack = np.zeros((nsteps, 128, 896), np.float32)
    wpack[:, 0:N, 0:H] = W1e
    wpack[:, :, 256:512] = W2[:, 0:128, :]
    wpack[:, :, 512:768] = W2[:, 128:256, :]
    wpack[:, :, 768:832] = W3e[:, 0:128, :]
    wpack[:, :, 832:896] = W3e[:, 128:256, :]
    b1r = np.ascontiguousarray(
        b1[:nsteps].reshape(nsteps, 2, 128).transpose(2, 0, 1).reshape(128, 2 * nsteps)
    )
    b2r = np.ascontiguousarray(
        b2[:nsteps].reshape(nsteps, 2, 128).transpose(2, 0, 1).reshape(128, 2 * nsteps)
    )
    wpack = wpack.astype(ml_dtypes.bfloat16)
    xt = np.ascontiguousarray(x.T).astype(ml_dtypes.bfloat16)
    sv = np.ascontiguousarray(np.asarray(s, np.float32).reshape(N, 1))
    return dict(wpack=wpack, b1r=b1r, b2r=b2r, b3c=b3c, sv=sv), xt


def _kernel_v1(x, s, W1, b1, W2, b2, W3, b3, idx):
    global LAST_RESULT
    shared, xt = _host_prep_v1(x, s, W1, b1, W2, b2, W3, b3, idx)
    in_maps = []
    for c in range(NCORES):
        m = dict(shared)
        m["xt"] = np.ascontiguousarray(xt[:, c * BSH : (c + 1) * BSH])
        in_maps.append(m)
    if True not in _PROGRAM_V1:
        _PROGRAM_V1[True] = build_program_v1(use_bias=True)
    _ensure_ntff_hook()
    res = run_bass_kernel_spmd(
        _PROGRAM_V1[True], in_maps, core_ids=list(range(NCORES))
    )
    LAST_RESULT = res
    outs = [res.results[c]["out"] for c in range(NCORES)]
    return np.ascontiguousarray(
        np.concatenate([o.T for o in outs], axis=0), dtype=np.float32
    )
